# revision 13
# baseline (speedup 1.0000x reference)
"""Trainium2 Bass kernel for DFlashAttentionANECache (sparse_attention).

8-way tensor-parallel over heads: core c owns KV head c and Q heads
4c..4c+3.  Each core projects K/V for its head over all 2064 tokens,
LayerNorms + RoPEs Q/K, streams flash-attention over the full 32K cache,
and produces a partial o_proj output; the host sums the 8 partials and
assembles the updated cache.

Host-side layout prep (so the device never transposes bulk data):
  - cT tiles:  c = concat(x_ctx, x) padded to 2176 rows, re-tiled to
               (17, 2048, 128) so each tile is H-major (matmul lhsT).
  - kT cache:  kv_cache[0, c].T  -> (128, 32768), K head streamed
               column-chunks directly as QK rhs.
  - v cache:   kv_cache[1, c]    -> (32768, 128) natural (PV rhs).
  - wkT/wvT/wqT/woT: pre-transposed weight shards.
"""

import os
import sys
import numpy as np
from contextlib import ExitStack

H, L, S, D, NH, NKV, SL = 2048, 16, 2048, 128, 32, 8, 32768
T = S + L                      # 2064
REP = NH // NKV                # 4
HALF = D // 2
EPS = 1e-6
SCALE = float(D) ** -0.5
N_CORES = 8
NT = 17                        # 128-token tiles covering T (last tile 16 valid)
TPAD = NT * 128                # 2176
HCH = H // 128                 # 16 hidden chunks
CHUNK = 1024                   # attention token chunk
N_CHUNKS = SL // CHUNK         # 32

_compiled = {}


def _build(pos):
    import concourse.bass as bass
    import concourse.tile as tile
    from concourse import bacc, mybir
    from concourse.masks import make_identity

    fp32 = mybir.dt.float32
    AF = mybir.ActivationFunctionType
    ALU = mybir.AluOpType

    assert pos % CHUNK == 0 and 0 <= pos <= SL - T, (
        f"kernel compiled only for CHUNK-aligned current_pos, got {pos}"
    )
    # chunk classification
    c_new0 = pos // CHUNK                    # first chunk fully new
    n_full_new = (T // CHUNK)                # 2 full new chunks (2048 tokens)
    c_mixed = c_new0 + n_full_new            # 16 new + 1008 old tokens
    new_chunks = list(range(c_new0, c_new0 + n_full_new))
    dram_chunks = [c for c in range(N_CHUNKS)
                   if c not in new_chunks and c != c_mixed]

    nc = bacc.Bacc("TRN2", target_bir_lowering=False, debug=False,
                   enable_asserts=False, num_devices=N_CORES)

    ct = nc.dram_tensor("ct", [NT, H, 128], fp32, kind="ExternalInput").ap()
    ktc = nc.dram_tensor("ktc", [D, SL], fp32, kind="ExternalInput").ap()
    vc = nc.dram_tensor("vc", [SL, D], fp32, kind="ExternalInput").ap()
    wkt = nc.dram_tensor("wkt", [H, D], fp32, kind="ExternalInput").ap()
    wvt = nc.dram_tensor("wvt", [H, D], fp32, kind="ExternalInput").ap()
    wqt = nc.dram_tensor("wqt", [H, REP * D], fp32, kind="ExternalInput").ap()
    wot = nc.dram_tensor("wot", [REP * D, H], fp32, kind="ExternalInput").ap()
    cosk = nc.dram_tensor("cosk", [T, D], fp32, kind="ExternalInput").ap()
    sink = nc.dram_tensor("sink", [T, D], fp32, kind="ExternalInput").ap()
    cosq = nc.dram_tensor("cosq", [L, D], fp32, kind="ExternalInput").ap()
    sinq = nc.dram_tensor("sinq", [L, D], fp32, kind="ExternalInput").ap()
    qnw = nc.dram_tensor("qnw", [D], fp32, kind="ExternalInput").ap()
    knw = nc.dram_tensor("knw", [D], fp32, kind="ExternalInput").ap()
    knew = nc.dram_tensor("knew", [T, D], fp32, kind="ExternalOutput").ap()
    vnew = nc.dram_tensor("vnew", [T, D], fp32, kind="ExternalOutput").ap()
    yout = nc.dram_tensor("yout", [L, H], fp32, kind="ExternalOutput").ap()

    def bcast(ap1d, parts):
        return bass.AP(tensor=ap1d.tensor, offset=ap1d.offset,
                       ap=[[0, parts]] + list(ap1d.ap))

    with tile.TileContext(nc) as tc, ExitStack() as ctx:
        singles = ctx.enter_context(tc.tile_pool(name="singles", bufs=1))
        p2k = ctx.enter_context(tc.tile_pool(name="p2k", bufs=3))
        p2v = ctx.enter_context(tc.tile_pool(name="p2v", bufs=3))
        p2e = ctx.enter_context(tc.tile_pool(name="p2e", bufs=2))
        p2s = ctx.enter_context(
            tc.tile_pool(name="p2s", bufs=2, space="PSUM"))
        p2t = ctx.enter_context(
            tc.tile_pool(name="p2t", bufs=1, space="PSUM"))
        p2acc = ctx.enter_context(
            tc.tile_pool(name="p2acc", bufs=1, space="PSUM"))

        # ---- persistent SBUF ----
        id_sb = singles.tile([128, 128], fp32)
        make_identity(nc, id_sb[:])
        qnw_sb = singles.tile([128, D], fp32)
        nc.gpsimd.dma_start(out=qnw_sb[:], in_=bcast(qnw, 128))
        knw_sb = singles.tile([128, D], fp32)
        nc.gpsimd.dma_start(out=knw_sb[:], in_=bcast(knw, 128))
        cosq_sb = singles.tile([L, D], fp32)
        nc.gpsimd.dma_start(out=cosq_sb[:], in_=cosq[:])
        sinq_sb = singles.tile([L, D], fp32)
        nc.gpsimd.dma_start(out=sinq_sb[:], in_=sinq[:])
        eps_sb = singles.tile([128, 1], fp32)
        nc.vector.memset(eps_sb[:], EPS)

        wk_sb = singles.tile([128, HCH, D], fp32)
        nc.sync.dma_start(out=wk_sb[:], in_=wkt.rearrange("(h p) d -> p h d", p=128))
        wv_sb = singles.tile([128, HCH, D], fp32)
        nc.sync.dma_start(out=wv_sb[:], in_=wvt.rearrange("(h p) d -> p h d", p=128))
        wq_sb = singles.tile([128, HCH, REP * D], fp32)
        nc.sync.dma_start(out=wq_sb[:], in_=wqt.rearrange("(h p) d -> p h d", p=128))

        kt_new = singles.tile([128, TPAD], fp32)       # post-rope K^T, new tokens
        v_new = singles.tile([128, NT, 128], fp32)     # V natural, new tokens
        qt_sb = singles.tile([128, REP * L], fp32)     # post-rope Q^T
        sums_sb = singles.tile([REP * L, 2 * N_CHUNKS], fp32)

        # ---- phase-1 pools ----
        p1ctx = ExitStack()
        p1ct = p1ctx.enter_context(tc.tile_pool(name="p1ct", bufs=2))
        p1w = p1ctx.enter_context(tc.tile_pool(name="p1w", bufs=3))
        p1cs = p1ctx.enter_context(tc.tile_pool(name="p1cs", bufs=2))
        # kp and vp must live in DIFFERENT psum banks: a matmul with
        # start=True marks its whole 2KB zero-region (bank) pending-zero,
        # so interleaved k/v accumulation groups sharing a bank corrupt
        # each other.
        p1kv = p1ctx.enter_context(
            tc.tile_pool(name="p1kv", bufs=1, space="PSUM"))
        p1ps = p1ctx.enter_context(
            tc.tile_pool(name="p1ps", bufs=2, space="PSUM"))

        def ln_rope_tile(src_ps, parts, w_sb, cos_sb, sin_sb, out_sb):
            """(x-mean)*rsqrt(var+eps)*w then rope, into out_sb[:parts]."""
            stats = p1w.tile([128, 6], fp32, tag="stats")
            mv = p1w.tile([128, 2], fp32, tag="mv")
            nc.vector.bn_stats(out=stats[:parts], in_=src_ps[:parts])
            nc.vector.bn_aggr(out=mv[:parts], in_=stats[:parts])
            rstd = p1w.tile([128, 1], fp32, tag="rstd")
            nc.scalar.activation(out=rstd[:parts], in_=mv[:parts, 1:2],
                                 func=AF.Sqrt, bias=eps_sb[:parts])
            nc.vector.reciprocal(out=rstd[:parts], in_=rstd[:parts])
            xln = p1w.tile([128, D], fp32, tag="xln")
            nc.vector.tensor_scalar(out=xln[:parts], in0=src_ps[:parts],
                                    scalar1=mv[:parts, 0:1],
                                    scalar2=rstd[:parts, 0:1],
                                    op0=ALU.subtract, op1=ALU.mult)
            nc.vector.tensor_mul(out=xln[:parts], in0=xln[:parts],
                                 in1=w_sb[:parts])
            t1 = p1w.tile([128, D], fp32, tag="t1")
            nc.vector.tensor_mul(out=t1[:parts], in0=xln[:parts],
                                 in1=cos_sb[:parts])
            t2 = p1w.tile([128, HALF], fp32, tag="t2")
            nc.vector.tensor_mul(out=t2[:parts], in0=xln[:parts, HALF:],
                                 in1=sin_sb[:parts, :HALF])
            nc.vector.tensor_sub(out=out_sb[:parts, :HALF],
                                 in0=t1[:parts, :HALF], in1=t2[:parts])
            nc.vector.tensor_mul(out=t2[:parts], in0=xln[:parts, :HALF],
                                 in1=sin_sb[:parts, HALF:])
            nc.vector.tensor_add(out=out_sb[:parts, HALF:],
                                 in0=t1[:parts, HALF:], in1=t2[:parts])

        def p1_tile(m, with_q):
            rows = min(128, T - m * 128)
            ct_t = p1ct.tile([128, HCH, 128], fp32, tag="ct")
            nc.sync.dma_start(out=ct_t[:],
                              in_=ct[m].rearrange("(h p) t -> p h t", p=128))
            kp = p1kv.tile([128, 128], fp32, tag="kp")
            vp = p1kv.tile([128, 128], fp32, tag="vp")
            if with_q:
                qp = p1ps.tile([L, REP * D], fp32, tag="misc")
            for h in range(HCH):
                lhs = ct_t[:, h, :]
                nc.tensor.matmul(kp[:], lhs, wk_sb[:, h, :],
                                 start=(h == 0), stop=(h == HCH - 1))
                nc.tensor.matmul(vp[:], lhs, wv_sb[:, h, :],
                                 start=(h == 0), stop=(h == HCH - 1))
                if with_q:
                    nc.tensor.matmul(qp[:], ct_t[:, h, 0:L], wq_sb[:, h, :],
                                     start=(h == 0), stop=(h == HCH - 1))

            # V: plain copy out of PSUM into persistent + DRAM
            nc.vector.tensor_copy(v_new[:, m, :], vp[:])
            nc.sync.dma_start(out=vnew[m * 128:m * 128 + rows, :],
                              in_=v_new[:rows, m, :])

            # K: LN + rope
            cos_t = p1cs.tile([128, D], fp32, tag="cos")
            sin_t = p1cs.tile([128, D], fp32, tag="sin")
            nc.gpsimd.dma_start(out=cos_t[:rows], in_=cosk[m * 128:m * 128 + rows])
            nc.gpsimd.dma_start(out=sin_t[:rows], in_=sink[m * 128:m * 128 + rows])
            kr = p1w.tile([128, D], fp32, tag="kr")
            ln_rope_tile(kp[:], rows, knw_sb, cos_t, sin_t, kr)
            nc.sync.dma_start(out=knew[m * 128:m * 128 + rows, :],
                              in_=kr[:rows])
            # transpose to kt_new columns
            ktp = p1ps.tile([128, 128], fp32, tag="misc")
            nc.tensor.transpose(ktp[:, :rows], kr[:rows], id_sb[:rows, :rows])
            nc.vector.tensor_copy(kt_new[:, m * 128:m * 128 + rows],
                                  ktp[:, :rows])

            if with_q:
                # Q: LN + rope per head, then transpose into qt_sb
                qr = p1w.tile([L, REP * D], fp32, tag="qr")
                for hh in range(REP):
                    sl = slice(hh * D, (hh + 1) * D)
                    ln_rope_tile(qp[:, sl], L, qnw_sb, cosq_sb, sinq_sb,
                                 qr[:, sl])
                qtp = p1ps.tile([128, REP * L], fp32, tag="misc")
                for hh in range(REP):
                    nc.tensor.transpose(qtp[:, hh * L:(hh + 1) * L],
                                        qr[:, hh * D:(hh + 1) * D],
                                        id_sb[:L, :L])
                nc.vector.tensor_copy(qt_sb[:], qtp[:])

        # ---- phase-2 chunk ----
        pv_ps = p2acc.tile([REP * L, D], fp32)
        pv_state = {"first": True}

        def p2_chunk(cix, is_last):
            base = cix * CHUNK
            if cix in new_chunks:
                off = base - pos
                kt_t = kt_new[:, off:off + CHUNK]
                v_t = v_new[:, off // 128:(off + CHUNK) // 128, :]
            elif cix == c_mixed:
                tail = T - n_full_new * CHUNK        # 16 new tokens
                kt_full = p2k.tile([128, CHUNK], fp32, tag="kt")
                nc.sync.dma_start(out=kt_full[:, tail:],
                                  in_=ktc[:, pos + T: base + CHUNK])
                nc.vector.tensor_copy(
                    kt_full[:, :tail],
                    kt_new[:, n_full_new * CHUNK:n_full_new * CHUNK + tail])
                v_full = p2v.tile([128, CHUNK // 128, 128], fp32, tag="v")
                nc.scalar.dma_start(
                    out=v_full[tail:, 0, :],
                    in_=vc[pos + T: base + 128, :])
                nc.scalar.dma_start(
                    out=v_full[:, 1:, :],
                    in_=vc[base + 128: base + CHUNK, :]
                        .rearrange("(t p) d -> p t d", p=128))
                nc.vector.tensor_copy(
                    v_full[:tail, 0, :],
                    v_new[:tail, (n_full_new * CHUNK) // 128, :])
                kt_t, v_t = kt_full, v_full
            else:
                kt_t = p2k.tile([128, CHUNK], fp32, tag="kt")
                nc.sync.dma_start(out=kt_t[:], in_=ktc[:, base: base + CHUNK])
                v_t = p2v.tile([128, CHUNK // 128, 128], fp32, tag="v")
                nc.scalar.dma_start(
                    out=v_t[:],
                    in_=vc[base: base + CHUNK].rearrange("(t p) d -> p t d", p=128))

            exp_sb = p2e.tile([REP * L, CHUNK], fp32, tag="exp")
            for half in range(CHUNK // 512):
                sp = p2s.tile([REP * L, 512], fp32, tag="sc")
                nc.tensor.matmul(sp[:], qt_sb[:],
                                 kt_t[:, half * 512:(half + 1) * 512],
                                 start=True, stop=True)
                col = 2 * cix + half
                nc.scalar.activation(
                    out=exp_sb[:, half * 512:(half + 1) * 512], in_=sp[:],
                    func=AF.Exp, scale=SCALE,
                    accum_out=sums_sb[:, col:col + 1])

            ntile = CHUNK // 128
            ept_ps = p2t.tile([128, ntile, 64], fp32, tag="ept")
            for t in range(ntile):
                nc.tensor.transpose(ept_ps[:, t, :],
                                    exp_sb[:, t * 128:(t + 1) * 128],
                                    id_sb[:REP * L, :REP * L])
            ept_sb = p2e.tile([128, ntile, 64], fp32, tag="ept_sb")
            nc.vector.tensor_copy(ept_sb[:], ept_ps[:])
            for t in range(ntile):
                nc.tensor.matmul(pv_ps[:], ept_sb[:, t, :], v_t[:, t, :],
                                 start=pv_state["first"],
                                 stop=(is_last and t == ntile - 1))
                pv_state["first"] = False

        # ---- emission order ----
        # Q tile first (gates all QK work), then interleave the remaining
        # projection tiles with DRAM attention chunks; on-chip (new-token)
        # chunks last since they need every projection tile.
        p1_tile(NT - 1, with_q=True)
        dq = list(dram_chunks) + [c_mixed]
        per = len(dq) / (NT - 1)
        taken = 0
        for m in range(NT - 1):
            p1_tile(m, with_q=False)
            want = int(round((m + 1) * per))
            while taken < want:
                p2_chunk(dq[taken], is_last=False)
                taken += 1
        while taken < len(dq):
            p2_chunk(dq[taken], is_last=False)
            taken += 1
        p1ctx.close()
        for c in new_chunks:
            p2_chunk(c, is_last=(c == new_chunks[-1]))

        # ---- epilogue ----
        ep = ExitStack()
        epw = ep.enter_context(tc.tile_pool(name="epw", bufs=1))
        epp = ep.enter_context(tc.tile_pool(name="epp", bufs=2, space="PSUM"))
        wo_sb = epw.tile([128, REP, H], fp32)
        nc.sync.dma_start(out=wo_sb[:], in_=wot.rearrange("(h p) j -> p h j", p=128))

        tot = epw.tile([REP * L, 1], fp32)
        nc.vector.tensor_reduce(out=tot[:], in_=sums_sb[:],
                                axis=mybir.AxisListType.X, op=ALU.add)
        nc.vector.reciprocal(out=tot[:], in_=tot[:])
        attn = epw.tile([REP * L, D], fp32)
        nc.vector.tensor_scalar_mul(attn[:], pv_ps[:], tot[:, 0:1])
        att_ps = epp.tile([128, REP * L], fp32, tag="att")
        nc.tensor.transpose(att_ps[:], attn[:], id_sb[:REP * L, :REP * L])
        att_sb = epw.tile([128, REP, L], fp32)
        nc.vector.tensor_copy(att_sb[:], att_ps[:].rearrange("p (h l) -> p h l", h=REP))
        y_sb = epw.tile([L, H], fp32)
        for n in range(4):
            y_ps = epp.tile([L, 512], fp32, tag="y")
            for hh in range(REP):
                nc.tensor.matmul(y_ps[:], att_sb[:, hh, :],
                                 wo_sb[:, hh, n * 512:(n + 1) * 512],
                                 start=(hh == 0), stop=(hh == REP - 1))
            nc.vector.tensor_copy(y_sb[:, n * 512:(n + 1) * 512], y_ps[:])
        nc.sync.dma_start(out=yout[:], in_=y_sb[:])
        ep.close()

    nc.compile()
    return nc


def _get_compiled(pos):
    if pos not in _compiled:
        _compiled[pos] = _build(pos)
    return _compiled[pos]


LAST_RESULTS = None


def kernel(x, x_ctx, cos_q, sin_q, cos_k, sin_k, kv_cache, causal_mask,
           wq, wk, wv, wo, qn_w, kn_w, current_pos):
    from concourse.bass_utils import run_bass_kernel_spmd

    global LAST_RESULTS
    pos = int(current_pos)
    f32 = np.float32

    x = np.asarray(x, f32)
    x_ctx = np.asarray(x_ctx, f32)
    kv_cache = np.asarray(kv_cache, f32)
    c = np.concatenate([x_ctx[0], x[0]], axis=0)          # (T, H)
    cpad = np.zeros((TPAD, H), f32)
    cpad[:T] = c
    ct3 = np.ascontiguousarray(cpad.reshape(NT, 128, H).transpose(0, 2, 1))

    cosk = np.asarray(cos_k, f32)
    sink = np.asarray(sin_k, f32)
    cosq = np.asarray(cos_q, f32)
    sinq = np.asarray(sin_q, f32)
    wq = np.asarray(wq, f32)
    wk = np.asarray(wk, f32)
    wv = np.asarray(wv, f32)
    wo = np.asarray(wo, f32)

    in_maps = []
    for cid in range(N_CORES):
        in_maps.append({
            "ct": ct3,
            "ktc": np.ascontiguousarray(kv_cache[0, cid].T),
            "vc": np.ascontiguousarray(kv_cache[1, cid]),
            "wkt": np.ascontiguousarray(wk[cid * D:(cid + 1) * D].T),
            "wvt": np.ascontiguousarray(wv[cid * D:(cid + 1) * D].T),
            "wqt": np.ascontiguousarray(
                wq[cid * REP * D:(cid + 1) * REP * D].T),
            "wot": np.ascontiguousarray(
                wo[:, cid * REP * D:(cid + 1) * REP * D].T),
            "cosk": cosk, "sink": sink, "cosq": cosq, "sinq": sinq,
            "qnw": np.asarray(qn_w, f32), "knw": np.asarray(kn_w, f32),
        })

    nc = _get_compiled(pos)
    res = run_bass_kernel_spmd(nc, in_maps, list(range(N_CORES)),
                               trace=False)
    LAST_RESULTS = res

    y = np.zeros((L, H), f32)
    out_cache = kv_cache.copy()
    for cid in range(N_CORES):
        r = res.results[cid]
        y += r["yout"]
        out_cache[0, cid, pos:pos + T] = r["knew"]
        out_cache[1, cid, pos:pos + T] = r["vnew"]
    return y.reshape(1, L, H), out_cache


# revision 22
# speedup vs baseline: 1.2988x; 1.2988x over previous
"""Trainium2 Bass kernel for DFlashAttentionANECache (sparse_attention).

8-way tensor-parallel over heads: core c owns KV head c and Q heads
4c..4c+3.  Each core projects K/V for its head over all 2064 tokens,
LayerNorms + RoPEs Q/K, streams attention over the full 32K cache, and
produces a partial o_proj output; the host sums the 8 partials and
assembles the updated cache.

Precision strategy: large matmuls run in float32r (single-pass PE,
~1.4e-4 rel err) — projections, QK, o_proj; the PV path (N=128, where
f32r is slow) runs in fp16 (exp weights are positive O(1..150), v is
O(0.02), ~5e-4).  Elementwise math (LN, rope, softmax denominators)
stays fp32.  Expected end-to-end rel err ~5e-4 vs the fp32 reference.

Host-side layout prep (device never transposes bulk data):
  cT tiles (17, 2048, 128); kT cache (128, 32768); v cache natural;
  pre-transposed weight shards, with wk/wv concatenated to (2048, 256).
"""

import numpy as np
from contextlib import ExitStack

H, L, S, D, NH, NKV, SL = 2048, 16, 2048, 128, 32, 8, 32768
T = S + L                      # 2064
REP = NH // NKV                # 4
HALF = D // 2
EPS = 1e-6
SCALE = float(D) ** -0.5
N_CORES = 8
NT = 17                        # 128-token tiles covering T (last tile 16 valid)
TPAD = NT * 128                # 2176
HCH = H // 128                 # 16 hidden chunks
CHUNK = 1024                   # attention token chunk
N_CHUNKS = SL // CHUNK         # 32

_compiled = {}


def _build(pos):
    import concourse.bass as bass
    import concourse.tile as tile
    from concourse import bacc, mybir
    from concourse.masks import make_identity

    fp32 = mybir.dt.float32
    f32r = mybir.dt.float32r
    fp16 = mybir.dt.float16
    AF = mybir.ActivationFunctionType
    ALU = mybir.AluOpType

    assert pos % CHUNK == 0 and 0 <= pos <= SL - T, (
        f"kernel compiled only for CHUNK-aligned current_pos, got {pos}"
    )
    c_new0 = pos // CHUNK
    n_full_new = T // CHUNK                  # 2 full new chunks
    c_mixed = c_new0 + n_full_new            # 16 new + 1008 old tokens
    new_chunks = list(range(c_new0, c_new0 + n_full_new))

    nc = bacc.Bacc("TRN2", target_bir_lowering=False, debug=False,
                   enable_asserts=False, num_devices=N_CORES)

    ct = nc.dram_tensor("ct", [NT, H, 128], f32r, kind="ExternalInput").ap()
    ktc = nc.dram_tensor("ktc", [D, SL], f32r, kind="ExternalInput").ap()
    vc = nc.dram_tensor("vc", [SL, D], fp32, kind="ExternalInput").ap()
    wkvt = nc.dram_tensor("wkvt", [H, 2 * D], f32r, kind="ExternalInput").ap()
    wqt = nc.dram_tensor("wqt", [H, REP * D], f32r, kind="ExternalInput").ap()
    wot = nc.dram_tensor("wot", [REP * D, H], f32r, kind="ExternalInput").ap()
    cosk = nc.dram_tensor("cosk", [T, D], fp32, kind="ExternalInput").ap()
    sink = nc.dram_tensor("sink", [T, D], fp32, kind="ExternalInput").ap()
    cosq = nc.dram_tensor("cosq", [L, D], fp32, kind="ExternalInput").ap()
    sinq = nc.dram_tensor("sinq", [L, D], fp32, kind="ExternalInput").ap()
    qnw = nc.dram_tensor("qnw", [D], fp32, kind="ExternalInput").ap()
    knw = nc.dram_tensor("knw", [D], fp32, kind="ExternalInput").ap()
    knew = nc.dram_tensor("knew", [T, D], fp32, kind="ExternalOutput").ap()
    vnew = nc.dram_tensor("vnew", [T, D], fp32, kind="ExternalOutput").ap()
    yout = nc.dram_tensor("yout", [L, H], fp32, kind="ExternalOutput").ap()

    def bcast(ap1d, parts):
        return bass.AP(tensor=ap1d.tensor, offset=ap1d.offset,
                       ap=[[0, parts]] + list(ap1d.ap))

    NSTAT = NT + REP  # 17 k-tile stat cols + 4 q-head stat cols

    with tile.TileContext(nc) as tc, ExitStack() as ctx:
        singles = ctx.enter_context(tc.tile_pool(name="singles", bufs=1))

        # ---- persistent SBUF ----
        id_sb = singles.tile([128, 128], fp32)
        make_identity(nc, id_sb[:])
        id16 = singles.tile([64, 64], fp16)
        nc.vector.tensor_copy(id16[:], id_sb[:64, :64])
        qnw_sb = singles.tile([128, D], fp32)
        nc.gpsimd.dma_start(out=qnw_sb[:], in_=bcast(qnw, 128))
        knw_sb = singles.tile([128, D], fp32)
        nc.gpsimd.dma_start(out=knw_sb[:], in_=bcast(knw, 128))
        cosq_sb = singles.tile([L, D], fp32)
        nc.gpsimd.dma_start(out=cosq_sb[:], in_=cosq[:])
        sinq_sb = singles.tile([L, D], fp32)
        nc.gpsimd.dma_start(out=sinq_sb[:], in_=sinq[:])
        eps_sb = singles.tile([128, 1], fp32)
        nc.vector.memset(eps_sb[:], EPS)

        wkv_sb = singles.tile([128, HCH, 2 * D], f32r)
        nc.sync.dma_start(out=wkv_sb[:],
                          in_=wkvt.rearrange("(h p) d -> p h d", p=128))
        wq_sb = singles.tile([128, HCH, REP * D], f32r)
        nc.scalar.dma_start(out=wq_sb[:],
                            in_=wqt.rearrange("(h p) d -> p h d", p=128))

        kt_new = singles.tile([128, TPAD], f32r)   # post-rope K^T, new tokens
        v_new16 = singles.tile([128, NT, 128], fp16)  # V (fp16), new tokens
        k_raw = singles.tile([128, NT, 128], fp32)    # pre-LN K
        q_raw = singles.tile([L, REP * D], fp32)
        qt_sb = singles.tile([128, REP * L], f32r)    # post-rope Q^T
        sums_sb = singles.tile([REP * L, 2 * N_CHUNKS], fp32)
        mv_all = singles.tile([128, NSTAT, 2], fp32)  # (mean, var) per tile
        rstd_all = singles.tile([128, NSTAT], fp32)
        nc.vector.memset(mv_all[:], 0.0)

        # ================= phase 1A: projections + stats =================
        p1ctx = ExitStack()
        p1ct = p1ctx.enter_context(tc.tile_pool(name="p1ct", bufs=3))
        p1w = p1ctx.enter_context(tc.tile_pool(name="p1w", bufs=3))
        p1cs = p1ctx.enter_context(tc.tile_pool(name="p1cs", bufs=2))
        p1kv = p1ctx.enter_context(
            tc.tile_pool(name="p1kv", bufs=2, space="PSUM"))
        p1qp = p1ctx.enter_context(
            tc.tile_pool(name="p1qp", bufs=1, space="PSUM"))
        p1tp = p1ctx.enter_context(
            tc.tile_pool(name="p1tp", bufs=2, space="PSUM"))

        def p1a_tile(m, with_q):
            rows = min(128, T - m * 128)
            ct_t = p1ct.tile([128, HCH, 128], f32r, tag="ct")
            nc.sync.dma_start(out=ct_t[:],
                              in_=ct[m].rearrange("(h p) t -> p h t", p=128))
            kv_ps = p1kv.tile([128, 2 * D], fp32, tag="kv")
            if with_q:
                qp = p1qp.tile([L, REP * D], fp32, tag="qp")
            for h in range(HCH):
                nc.tensor.matmul(kv_ps[:], ct_t[:, h, :], wkv_sb[:, h, :],
                                 start=(h == 0), stop=(h == HCH - 1))
                if with_q:
                    nc.tensor.matmul(qp[:], ct_t[:, h, 0:L], wq_sb[:, h, :],
                                     start=(h == 0), stop=(h == HCH - 1))
            stats = p1w.tile([128, 6], fp32, tag="stats")
            nc.vector.bn_stats(out=stats[:], in_=kv_ps[:, 0:D])
            nc.vector.bn_aggr(out=mv_all[:, m, :], in_=stats[:])
            nc.vector.tensor_copy(k_raw[:, m, :], kv_ps[:, 0:D])
            nc.vector.tensor_copy(v_new16[:, m, :], kv_ps[:, D:2 * D])
            vout = p1w.tile([128, D], fp32, tag="vout")
            nc.vector.tensor_copy(vout[:], kv_ps[:, D:2 * D])
            nc.scalar.dma_start(out=vnew[m * 128:m * 128 + rows, :],
                                in_=vout[:rows])
            if with_q:
                for hh in range(REP):
                    qst = p1w.tile([L, 6], fp32, tag="qst")
                    nc.vector.bn_stats(out=qst[:],
                                       in_=qp[:, hh * D:(hh + 1) * D])
                    nc.vector.bn_aggr(out=mv_all[0:L, NT + hh, :], in_=qst[:])
                nc.vector.tensor_copy(q_raw[:], qp[:])

        p1a_tile(NT - 1, with_q=True)
        for m in range(NT - 1):
            p1a_tile(m, with_q=False)

        # batched rstd: one Sqrt pass for all tiles (avoids ACT Exp<->Sqrt
        # table thrash during the attention stream)
        std_in = mv_all[:].rearrange("p a b -> p (a b)")[:, 1::2]
        nc.scalar.activation(out=rstd_all[:], in_=std_in,
                             func=AF.Sqrt, bias=eps_sb[:])
        nc.vector.reciprocal(out=rstd_all[:], in_=rstd_all[:])

        # ================= phase 1B: LN + rope + transposes ==============
        def ln_rope(src, parts, mcol, w_sb, cos_sb, sin_sb, out_sb):
            xln = p1w.tile([128, D], fp32, tag="xln")
            nc.vector.tensor_scalar(out=xln[:parts], in0=src[0:parts],
                                    scalar1=mv_all[0:parts, mcol, 0:1],
                                    scalar2=rstd_all[0:parts, mcol:mcol + 1],
                                    op0=ALU.subtract, op1=ALU.mult)
            nc.vector.tensor_mul(out=xln[:parts], in0=xln[:parts],
                                 in1=w_sb[:parts])
            t1 = p1w.tile([128, D], fp32, tag="t1")
            nc.vector.tensor_mul(out=t1[:parts], in0=xln[:parts],
                                 in1=cos_sb[:parts])
            t2 = p1w.tile([128, HALF], fp32, tag="t2")
            nc.vector.tensor_mul(out=t2[:parts], in0=xln[:parts, HALF:],
                                 in1=sin_sb[:parts, :HALF])
            nc.vector.tensor_sub(out=out_sb[:parts, :HALF],
                                 in0=t1[:parts, :HALF], in1=t2[:parts])
            nc.vector.tensor_mul(out=t2[:parts], in0=xln[:parts, :HALF],
                                 in1=sin_sb[:parts, HALF:])
            nc.vector.tensor_add(out=out_sb[:parts, HALF:],
                                 in0=t1[:parts, HALF:], in1=t2[:parts])

        # Q first (it gates all QK matmuls)
        qr = p1w.tile([L, REP * D], fp32, tag="qr")
        for hh in range(REP):
            sl = slice(hh * D, (hh + 1) * D)
            ln_rope(q_raw[:, sl], L, NT + hh, qnw_sb, cosq_sb, sinq_sb,
                    qr[:, sl])
        qtp = p1tp.tile([128, REP * L], fp32, tag="tp")
        for hh in range(REP):
            nc.tensor.transpose(qtp[:, hh * L:(hh + 1) * L],
                                qr[:, hh * D:(hh + 1) * D], id_sb[:L, :L])
        nc.vector.tensor_copy(qt_sb[:], qtp[:])

        for m in range(NT):
            rows = min(128, T - m * 128)
            cos_t = p1cs.tile([128, D], fp32, tag="cos")
            sin_t = p1cs.tile([128, D], fp32, tag="sin")
            nc.gpsimd.dma_start(out=cos_t[:rows],
                                in_=cosk[m * 128:m * 128 + rows])
            nc.gpsimd.dma_start(out=sin_t[:rows],
                                in_=sink[m * 128:m * 128 + rows])
            kr = p1w.tile([128, D], fp32, tag="kr")
            ln_rope(k_raw[:, m, :], rows, m, knw_sb, cos_t, sin_t, kr)
            nc.sync.dma_start(out=knew[m * 128:m * 128 + rows, :],
                              in_=kr[:rows])
            ktp = p1tp.tile([128, 128], fp32, tag="tp")
            nc.tensor.transpose(ktp[:, :rows], kr[:rows], id_sb[:rows, :rows])
            nc.vector.tensor_copy(kt_new[:, m * 128:m * 128 + rows],
                                  ktp[:, :rows])

        p1ctx.close()

        # ================= phase 2: attention stream =====================
        p2k = ctx.enter_context(tc.tile_pool(name="p2k", bufs=3))
        p2vs = ctx.enter_context(tc.tile_pool(name="p2vs", bufs=2))
        p2v = ctx.enter_context(tc.tile_pool(name="p2v", bufs=3))
        p2e = ctx.enter_context(tc.tile_pool(name="p2e", bufs=2))
        p2s = ctx.enter_context(
            tc.tile_pool(name="p2s", bufs=2, space="PSUM"))
        p2t = ctx.enter_context(
            tc.tile_pool(name="p2t", bufs=2, space="PSUM"))
        p2acc = ctx.enter_context(
            tc.tile_pool(name="p2acc", bufs=1, space="PSUM"))
        pv_ps = p2acc.tile([REP * L, D], fp32)
        ntile = CHUNK // 128

        wo_sb = singles.tile([128, REP, H], f32r)  # DMA emitted mid-stream

        def p2_chunk(cix, is_first, is_last):
            # pieces: (col_start, col_end, v_source_ap) with all partition
            # accesses base-0
            base = cix * CHUNK
            pieces = None
            if cix in new_chunks:
                off = base - pos
                kt_t = kt_new[:, off:off + CHUNK]
                v16 = v_new16[:, off // 128:(off + CHUNK) // 128, :]
            elif cix == c_mixed:
                # 16 new tokens | 112 old | 7x128 old — sub-tiles split at
                # the cache-update boundary so every access is partition-
                # base-0.
                tail = T - n_full_new * CHUNK        # 16 new tokens
                kt_full = p2k.tile([128, CHUNK], f32r, tag="kt")
                nc.sync.dma_start(out=kt_full[:, tail:],
                                  in_=ktc[:, pos + T: base + CHUNK])
                nc.vector.tensor_copy(
                    kt_full[:, :tail],
                    kt_new[:, n_full_new * CHUNK:n_full_new * CHUNK + tail])
                vst = p2vs.tile([128, ntile, 128], fp32, tag="vs")
                nc.scalar.dma_start(
                    out=vst[:128 - tail, 0, :], in_=vc[pos + T: base + 128, :])
                nc.scalar.dma_start(
                    out=vst[:, 1:, :],
                    in_=vc[base + 128: base + CHUNK, :]
                        .rearrange("(t p) d -> p t d", p=128))
                vt = p2v.tile([128, ntile, 128], fp16, tag="v")
                nc.vector.tensor_copy(vt[:, 1:, :], vst[:, 1:, :])
                nc.vector.tensor_copy(vt[:128 - tail, 0, :],
                                      vst[:128 - tail, 0, :])
                kt_t = kt_full
                v16 = None
                pieces = ([(0, tail,
                            v_new16[:tail, (n_full_new * CHUNK) // 128, :]),
                           (tail, 128, vt[:128 - tail, 0, :])] +
                          [(128 * (t + 1), 128 * (t + 2), vt[:, t + 1, :])
                           for t in range(ntile - 1)])
            else:
                kt_t = p2k.tile([128, CHUNK], f32r, tag="kt")
                nc.sync.dma_start(out=kt_t[:], in_=ktc[:, base: base + CHUNK])
                vst = p2vs.tile([128, ntile, 128], fp32, tag="vs")
                nc.scalar.dma_start(
                    out=vst[:],
                    in_=vc[base: base + CHUNK].rearrange("(t p) d -> p t d",
                                                         p=128))
                v16 = p2v.tile([128, ntile, 128], fp16, tag="v")
                nc.vector.tensor_copy(v16[:], vst[:])

            exp16 = p2e.tile([REP * L, CHUNK], fp16, tag="exp")
            for half in range(CHUNK // 512):
                sp = p2s.tile([REP * L, 512], fp32, tag="sc")
                nc.tensor.matmul(sp[:], qt_sb[:],
                                 kt_t[:, half * 512:(half + 1) * 512],
                                 start=True, stop=True)
                col = 2 * cix + half
                nc.scalar.activation(
                    out=exp16[:, half * 512:(half + 1) * 512], in_=sp[:],
                    func=AF.Exp, scale=SCALE,
                    accum_out=sums_sb[:, col:col + 1])
            if pieces is None:
                pieces = [(128 * t, 128 * (t + 1), v16[:, t, :])
                          for t in range(ntile)]
            np_ = len(pieces)
            ept_ps = p2t.tile([128, ntile + 1, 64], fp16, tag="ept")
            for t, (a, b, _) in enumerate(pieces):
                nc.tensor.transpose(ept_ps[:b - a, t, :],
                                    exp16[:, a:b], id16[:])
            ept16 = p2e.tile([128, ntile + 1, 64], fp16, tag="ept_sb")
            nc.vector.tensor_copy(ept16[:, :np_, :], ept_ps[:, :np_, :])
            for t, (a, b, v_ap) in enumerate(pieces):
                nc.tensor.matmul(pv_ps[:], ept16[:b - a, t, :], v_ap,
                                 start=(is_first and t == 0),
                                 stop=(is_last and t == np_ - 1))

        for i, cix in enumerate(range(N_CHUNKS)):
            if cix == N_CHUNKS - 8:
                nc.sync.dma_start(
                    out=wo_sb[:],
                    in_=wot.rearrange("(h p) j -> p h j", p=128))
            p2_chunk(cix, is_first=(cix == 0), is_last=(cix == N_CHUNKS - 1))

        # ================= epilogue: softmax scale + o_proj ==============
        epw = ctx.enter_context(tc.tile_pool(name="epw", bufs=1))
        epp = ctx.enter_context(tc.tile_pool(name="epp", bufs=1, space="PSUM"))
        tot = epw.tile([REP * L, 1], fp32)
        nc.vector.tensor_reduce(out=tot[:], in_=sums_sb[:],
                                axis=mybir.AxisListType.X, op=ALU.add)
        nc.vector.reciprocal(out=tot[:], in_=tot[:])
        attn = epw.tile([REP * L, D], fp32)
        nc.vector.tensor_scalar_mul(attn[:], pv_ps[:], tot[:, 0:1])
        att_ps = epp.tile([128, REP * L], fp32, tag="att")
        nc.tensor.transpose(att_ps[:], attn[:], id_sb[:REP * L, :REP * L])
        att_sb = epw.tile([128, REP, L], f32r)
        nc.vector.tensor_copy(att_sb[:],
                              att_ps[:].rearrange("p (h l) -> p h l", h=REP))
        y_sb = epw.tile([L, H], fp32)
        for n in range(4):
            y_ps = epp.tile([L, 512], fp32, tag="y")
            for hh in range(REP):
                nc.tensor.matmul(y_ps[:], att_sb[:, hh, :],
                                 wo_sb[:, hh, n * 512:(n + 1) * 512],
                                 start=(hh == 0), stop=(hh == REP - 1))
            nc.vector.tensor_copy(y_sb[:, n * 512:(n + 1) * 512], y_ps[:])
        nc.sync.dma_start(out=yout[:], in_=y_sb[:])

    nc.compile()
    return nc


def _get_compiled(pos):
    if pos not in _compiled:
        _compiled[pos] = _build(pos)
    return _compiled[pos]


LAST_RESULTS = None


def kernel(x, x_ctx, cos_q, sin_q, cos_k, sin_k, kv_cache, causal_mask,
           wq, wk, wv, wo, qn_w, kn_w, current_pos):
    from concourse.bass_utils import run_bass_kernel_spmd

    global LAST_RESULTS
    pos = int(current_pos)
    f32 = np.float32

    x = np.asarray(x, f32)
    x_ctx = np.asarray(x_ctx, f32)
    kv_cache = np.asarray(kv_cache, f32)
    c = np.concatenate([x_ctx[0], x[0]], axis=0)          # (T, H)
    cpad = np.zeros((TPAD, H), f32)
    cpad[:T] = c
    ct3 = np.ascontiguousarray(cpad.reshape(NT, 128, H).transpose(0, 2, 1))

    wq = np.asarray(wq, f32)
    wk = np.asarray(wk, f32)
    wv = np.asarray(wv, f32)
    wo = np.asarray(wo, f32)

    in_maps = []
    for cid in range(N_CORES):
        wkvt = np.concatenate([wk[cid * D:(cid + 1) * D].T,
                               wv[cid * D:(cid + 1) * D].T], axis=1)
        in_maps.append({
            "ct": ct3,
            "ktc": np.ascontiguousarray(kv_cache[0, cid].T),
            "vc": np.ascontiguousarray(kv_cache[1, cid]),
            "wkvt": np.ascontiguousarray(wkvt),
            "wqt": np.ascontiguousarray(
                wq[cid * REP * D:(cid + 1) * REP * D].T),
            "wot": np.ascontiguousarray(
                wo[:, cid * REP * D:(cid + 1) * REP * D].T),
            "cosk": np.asarray(cos_k, f32), "sink": np.asarray(sin_k, f32),
            "cosq": np.asarray(cos_q, f32), "sinq": np.asarray(sin_q, f32),
            "qnw": np.asarray(qn_w, f32), "knw": np.asarray(kn_w, f32),
        })

    nc = _get_compiled(pos)
    res = run_bass_kernel_spmd(nc, in_maps, list(range(N_CORES)),
                               trace=False)
    LAST_RESULTS = res

    y = np.zeros((L, H), f32)
    out_cache = kv_cache.copy()
    for cid in range(N_CORES):
        r = res.results[cid]
        y += r["yout"]
        out_cache[0, cid, pos:pos + T] = r["knew"]
        out_cache[1, cid, pos:pos + T] = r["vnew"]
    return y.reshape(1, L, H), out_cache
